# revision 6
# baseline (speedup 1.0000x reference)
"""Trainium2 Bass kernel for nn_Eq1to3 (gnn_message_passing).

Reference computation:
    Y  = einsum('ndi,dsb->nsbi', x, coefs[:, :, :3])      # (n, s, 3, m)
    S  = einsum('nd,ds->ns', x.sum(-1), coefs[:, :, 3])   # (n, s)
    out[n,s,i,j,k] = Y0[n,s,i] + Y1[n,s,j] + Y2[n,s,k] + S[n,s] + bias[s]

Shapes: x (4, 16, 96) f32  ->  out (4, 16, 96, 96, 96) f32 (~226.5 MB).
The contractions are tiny; the work is materializing + writing 226 MB.
Entirely HBM-write bound on device.

Strategy (8 NeuronCores):
  - Shard (n, i): core c handles n = c//2 and i in [48*(c%2), 48*(c%2)+48).
    Per-core output: (16, 48, 96, 96) = 28.3 MB.
  - Host precomputes  W[n, s, (j,k)] = Y1[n,s,j] + Y2[n,s,k] + S[n,s] + bias[s]
    (independent of i!) and A[n,s,i] = Y0[n,s,i].
  - Device tile layout: 128 partitions = (s: 16) x (i-chunk: 8), free = (j,k)
    = 9216.  The same replicated W tile (row p holds W[n, p//8, :]) serves all
    six i-chunks; per chunk only a per-partition scalar A column is added.
  - Per i-chunk: one DVE/ACT tensor+scalar pass (SBUF->SBUF, 4.72 MB) and one
    4.72 MB dma_start to the per-core output slab (per-partition contiguous
    36.9 KB runs in HBM).
"""

import sys

sys.path.insert(0, "/opt/trn_rl_repo")

import numpy as np

import concourse.bass as bass
import concourse.bacc as bacc
import concourse.mybir as mybir
from concourse.tile import TileContext
from concourse.bass_utils import run_bass_kernel_spmd

N_BATCH = 4
IN_DIM = 16
OUT_DIM = 16
M = 96
JK = M * M  # 9216
N_CORES = 8
I_PER_CORE = 48  # one n, half of the i axis per core
I_CHUNK = 8  # 16 s * 8 i = 128 partitions
N_CHUNKS = I_PER_CORE // I_CHUNK  # 6
F_SPLIT = 4  # free-dim split of the add pass (pipelining granularity)

_PROGRAM_CACHE = {}


def _build_program():
    nc = bacc.Bacc(None)
    # Single input tensor: [:, :JK] = replicated W rows, [:, JK:] = per-chunk A
    # columns.  One DMA -> every downstream compute op waits on one sem lane
    # (the ACT instruction struct supports only a single sync-wait command).
    wa_d = nc.dram_tensor(
        "wa", [128, JK + N_CHUNKS], mybir.dt.float32, kind="ExternalInput"
    )
    o_d = nc.dram_tensor(
        "o", [OUT_DIM, I_PER_CORE, JK], mybir.dt.float32, kind="ExternalOutput"
    )

    with TileContext(nc) as tc:
        with (
            tc.tile_pool(name="wpool", bufs=1) as wpool,
            tc.tile_pool(name="bigpool", bufs=2) as bigpool,
        ):
            wa_sb = wpool.tile([128, JK + N_CHUNKS], mybir.dt.float32)
            nc.sync.dma_start(out=wa_sb[:], in_=wa_d[:])

            fs = JK // F_SPLIT
            for t in range(N_CHUNKS):
                big = bigpool.tile([128, JK], mybir.dt.float32)
                a_col = wa_sb[:, JK + t : JK + t + 1]
                # Whole tile on ONE engine so the out-DMA needs a single
                # sync-wait (the TRN2 instruction structs accept only one).
                for f in range(F_SPLIT):
                    sl = slice(f * fs, (f + 1) * fs)
                    if t % 2 == 0:
                        nc.vector.tensor_scalar_add(
                            out=big[:, sl], in0=wa_sb[:, sl], scalar1=a_col
                        )
                    else:
                        nc.scalar.add(big[:, sl], wa_sb[:, sl], a_col)
                nc.sync.dma_start(out=o_d[:, t * I_CHUNK : (t + 1) * I_CHUNK, :], in_=big[:]
                )

    nc.compile()
    return nc


def _host_precompute(x, coefs, bias):
    x = np.asarray(x, dtype=np.float32)
    coefs = np.asarray(coefs, dtype=np.float32)
    bias = np.asarray(bias, dtype=np.float32)

    # (n, s, 3, m) and (n, s)
    Y = np.einsum("ndi,dsb->nsbi", x, coefs[:, :, :3], optimize=True).astype(np.float32)
    S = np.einsum("nd,ds->ns", x.sum(axis=-1), coefs[:, :, 3], optimize=True).astype(
        np.float32
    )
    A = Y[:, :, 0, :]  # (n, s, i)
    Y1 = Y[:, :, 1, :]  # (n, s, j)
    Z2 = Y[:, :, 2, :] + (S + bias.reshape(1, OUT_DIM))[:, :, None]  # (n, s, k)
    # W[n, s, j*96+k] = Y1[n,s,j] + Z2[n,s,k]
    W = (Y1[:, :, :, None] + Z2[:, :, None, :]).reshape(N_BATCH, OUT_DIM, JK)
    return W.astype(np.float32), A.astype(np.float32)


def _make_in_maps(W, A):
    in_maps = []
    for c in range(N_CORES):
        n = c // 2
        i0 = (c % 2) * I_PER_CORE
        w_in = np.repeat(W[n], I_CHUNK, axis=0)  # (128, 9216), row p = W[n, p//8]
        a_in = (
            A[n, :, i0 : i0 + I_PER_CORE]
            .reshape(OUT_DIM, N_CHUNKS, I_CHUNK)
            .transpose(0, 2, 1)
            .reshape(128, N_CHUNKS)
        )
        wa = np.concatenate([w_in, a_in], axis=1)  # (128, JK + N_CHUNKS)
        in_maps.append({"wa": np.ascontiguousarray(wa)})
    return in_maps


def _run(inputs, trace=False, **kwargs):
    W, A = _host_precompute(inputs["x"], inputs["coefs"], inputs["bias"])
    if "nc" not in _PROGRAM_CACHE:
        _PROGRAM_CACHE["nc"] = _build_program()
    nc = _PROGRAM_CACHE["nc"]
    in_maps = _make_in_maps(W, A)
    res = run_bass_kernel_spmd(nc, in_maps, list(range(N_CORES)), trace=trace, **kwargs)

    out = np.empty((N_BATCH, OUT_DIM, M, M, M), dtype=np.float32)
    for c in range(N_CORES):
        n = c // 2
        i0 = (c % 2) * I_PER_CORE
        out[n, :, i0 : i0 + I_PER_CORE] = res.results[c]["o"].reshape(
            OUT_DIM, I_PER_CORE, M, M
        )
    return out, res


def kernel(**inputs) -> np.ndarray:
    out, _ = _run(inputs, trace=False)
    return out


if __name__ == "__main__":
    rng = np.random.default_rng(0)
    x = rng.standard_normal((N_BATCH, IN_DIM, M), dtype=np.float32)
    coefs = rng.standard_normal((IN_DIM, OUT_DIM, 4), dtype=np.float32)
    bias = np.zeros((1, OUT_DIM, 1, 1, 1), dtype=np.float32)
    out = kernel(x=x, coefs=coefs, bias=bias)
    print("kernel out", out.shape, out.dtype, float(np.abs(out).max()))


# revision 10
# speedup vs baseline: 3.3797x; 3.3797x over previous
"""Trainium2 Bass kernel for nn_Eq1to3 (gnn_message_passing).

Reference computation:
    Y  = einsum('ndi,dsb->nsbi', x, coefs[:, :, :3])      # (n, s, 3, m)
    S  = einsum('nd,ds->ns', x.sum(-1), coefs[:, :, 3])   # (n, s)
    out[n,s,i,j,k] = Y0[n,s,i] + Y1[n,s,j] + Y2[n,s,k] + S[n,s] + bias[s]

Shapes: x (4, 16, 96) f32 -> out (4, 16, 96, 96, 96) f32 (~226.5 MB).
The contractions are tiny (a few MFLOP); the real work is materializing and
writing 226 MB — the kernel is HBM-write bound.

Strategy (8 NeuronCores):
  * Shard (n, i): core c handles n = c//2, i in [48*(c%2), 48*(c%2)+48).
    Per-core output 28.3 MB — perfectly balanced, no collectives.
  * Host precomputes (microscopic contractions, fp32 exact):
        W[n, s, (j,k)] = Y1[n,s,j] + Y2[n,s,k] + S[n,s] + bias[s]   (i-free!)
        A[n, s, i]     = Y0[n,s,i]
  * Device tile layout: 128 partitions = (s: 16) x (i-chunk: 8), free dim =
    (j,k) = 9216.  One SBUF tile big0 holds W replicated 8x per s-row; it is
    built once from a tiny (128, 1152) packed W via 8 SBUF->SBUF DMAs with a
    zero-stride (broadcast) source access pattern.  The SAME big0 serves all
    six i-chunks — per chunk only a per-partition scalar column A changes.
  * Per i-chunk: 4 DVE tensor_scalar adds (big = big0 + a_t) and one 4.72 MB
    dma_start to a contiguous HBM block, alternating the two HWDGE rings
    (SP / ACT) for back-to-back queue drain.
  * Per-core HBM traffic = 28.3 MB out + 0.6 MB in  ->  ~80 us roofline at
    ~358 GB/s per-core HBM bandwidth.  All compute (DVE ~30 us) is hidden.
    fp32-exact end to end (no bf16): rel err vs fp32 reference ~2e-7.

The per-core output layout is chunk-major (t, s, i', j*96+k) so every DMA
destination is contiguous; the host gathers/permutes shards into the full
(4, 16, 96, 96, 96) array.
"""

import dataclasses
import sys

sys.path.insert(0, "/opt/trn_rl_repo")

import numpy as np

import concourse.bacc as bacc
import concourse.mybir as mybir
from concourse.tile import TileContext
from concourse.bass_utils import run_bass_kernel_spmd

N_BATCH = 4
IN_DIM = 16
OUT_DIM = 16
M = 96
JK = M * M  # 9216
N_CORES = 8
I_PER_CORE = 48  # one n, half of the i axis per core
I_CHUNK = 8  # 16 s * 8 i = 128 partitions
N_CHUNKS = I_PER_CORE // I_CHUNK  # 6
PITCH = JK // I_CHUNK  # 1152: packed-W row length
F_SPLIT = 4  # DVE op granularity (2304 cols per op)

_PROGRAM_CACHE = {}


def _build_program():
    nc = bacc.Bacc(None)
    # Packed W: row p = W[n, p//8, (p%8)*PITCH : (p%8+1)*PITCH]  (128, 1152)
    w_d = nc.dram_tensor("w", [128, PITCH], mybir.dt.float32, kind="ExternalInput")
    # A columns: a[p, t] = A value for partition p = (s, i') in i-chunk t
    a_d = nc.dram_tensor("a", [128, N_CHUNKS], mybir.dt.float32, kind="ExternalInput")
    o_d = nc.dram_tensor(
        "o", [N_CHUNKS, OUT_DIM, I_CHUNK, JK], mybir.dt.float32, kind="ExternalOutput"
    )

    with TileContext(nc) as tc:
        with (
            tc.tile_pool(name="spool", bufs=1) as spool,
            tc.tile_pool(name="b0pool", bufs=1) as b0pool,
            tc.tile_pool(name="bigpool", bufs=4) as bigpool,
        ):
            w_sb = spool.tile([128, PITCH], mybir.dt.float32)
            a_sb = spool.tile([128, N_CHUNKS], mybir.dt.float32)
            nc.sync.dma_start(out=w_sb[:], in_=w_d[:])
            nc.scalar.dma_start(out=a_sb[:], in_=a_d[:])

            big0 = b0pool.tile([128, JK], mybir.dt.float32)
            # Replicate: big0[p=(s,i'), e*PITCH+k'] = w_sb[s*8+e, k'] for all
            # i'.  Source AP [[PITCH*8, 16], [0, 8], [1, PITCH]] at offset
            # e*PITCH — SBUF APs use linearized partition-major addressing
            # (partition pitch = tile free size), so the zero-stride middle
            # dim broadcasts each source row to 8 destination partitions.
            for e in range(I_CHUNK):
                src = dataclasses.replace(
                    w_sb[:],
                    offset=e * PITCH,
                    ap=[[PITCH * I_CHUNK, OUT_DIM], [0, I_CHUNK], [1, PITCH]],
                )
                eng = nc.sync if e % 2 == 0 else nc.scalar
                eng.dma_start(out=big0[:, e * PITCH : (e + 1) * PITCH], in_=src)

            fs = JK // F_SPLIT
            for t in range(N_CHUNKS):
                big = bigpool.tile([128, JK], mybir.dt.float32)
                a_t = a_sb[:, t : t + 1]
                for f in range(F_SPLIT):
                    sl = slice(f * fs, (f + 1) * fs)
                    nc.vector.tensor_scalar_add(
                        out=big[:, sl], in0=big0[:, sl], scalar1=a_t
                    )
                eng = nc.sync if t % 2 == 0 else nc.scalar
                eng.dma_start(out=o_d[t], in_=big[:])

    nc.compile()
    return nc


def _host_precompute(x, coefs, bias):
    x = np.asarray(x, dtype=np.float32)
    coefs = np.asarray(coefs, dtype=np.float32)
    bias = np.asarray(bias, dtype=np.float32)
    Y = np.einsum("ndi,dsb->nsbi", x, coefs[:, :, :3], optimize=True).astype(np.float32)
    S = np.einsum("nd,ds->ns", x.sum(axis=-1), coefs[:, :, 3], optimize=True).astype(
        np.float32
    )
    A = Y[:, :, 0, :]  # (n, s, i)
    Y1 = Y[:, :, 1, :]  # (n, s, j)
    Z2 = Y[:, :, 2, :] + (S + bias.reshape(1, OUT_DIM))[:, :, None]  # (n, s, k)
    W = (Y1[:, :, :, None] + Z2[:, :, None, :]).reshape(N_BATCH, OUT_DIM, JK)
    return W.astype(np.float32), A.astype(np.float32)


def _make_in_maps(W, A):
    in_maps = []
    for c in range(N_CORES):
        n = c // 2
        i0 = (c % 2) * I_PER_CORE
        w128 = W[n].reshape(128, PITCH)
        a_in = (
            A[n, :, i0 : i0 + I_PER_CORE]
            .reshape(OUT_DIM, N_CHUNKS, I_CHUNK)
            .transpose(0, 2, 1)
            .reshape(128, N_CHUNKS)
        )
        in_maps.append(
            {"w": np.ascontiguousarray(w128), "a": np.ascontiguousarray(a_in)}
        )
    return in_maps


def _run(inputs, trace=False, **kwargs):
    W, A = _host_precompute(inputs["x"], inputs["coefs"], inputs["bias"])
    if "nc" not in _PROGRAM_CACHE:
        _PROGRAM_CACHE["nc"] = _build_program()
    nc = _PROGRAM_CACHE["nc"]
    in_maps = _make_in_maps(W, A)
    res = run_bass_kernel_spmd(nc, in_maps, list(range(N_CORES)), trace=trace, **kwargs)

    out = np.empty((N_BATCH, OUT_DIM, M, M, M), dtype=np.float32)
    for c in range(N_CORES):
        n = c // 2
        i0 = (c % 2) * I_PER_CORE
        blk = res.results[c]["o"].reshape(N_CHUNKS, OUT_DIM, I_CHUNK, M, M)
        out[n, :, i0 : i0 + I_PER_CORE] = blk.transpose(1, 0, 2, 3, 4).reshape(
            OUT_DIM, I_PER_CORE, M, M
        )
    return out, res


def kernel(**inputs) -> np.ndarray:
    out, _ = _run(inputs, trace=False)
    return out


if __name__ == "__main__":
    rng = np.random.default_rng(0)
    x = rng.standard_normal((N_BATCH, IN_DIM, M), dtype=np.float32)
    coefs = rng.standard_normal((IN_DIM, OUT_DIM, 4), dtype=np.float32)
    bias = np.zeros((1, OUT_DIM, 1, 1, 1), dtype=np.float32)
    out = kernel(x=x, coefs=coefs, bias=bias)
    # host reference for smoke check
    Y = np.einsum("ndi,dsb->nsbi", x, coefs[:, :, :3])
    S = np.einsum("nd,ds->ns", x.sum(-1), coefs[:, :, 3])
    exp = (
        Y[:, :, 0, :, None, None]
        + Y[:, :, 1, None, :, None]
        + Y[:, :, 2, None, None, :]
        + S[:, :, None, None, None]
    )
    print("smoke max err:", float(np.abs(out - exp).max()))
